# revision 11
# baseline (speedup 1.0000x reference)
"""Multi-head attention (b=4, l=2048, d=1024, h=16) on 8 TRN2 NeuronCores.

Sharding: batch (4-way) x query-sequence (2-way) => 8 shards, no collectives.
Each core computes, for its (batch, query-half):
  - K/V projections for the full 2048-key sequence (duplicated across the
    2 cores sharing a batch), Q projection for its 1024 queries.
  - Scores in transposed orientation sT[k, q] = (k_h q_h^T) so the PV matmul
    contracts k on partitions; softmax without max-subtraction (scores ~N(0,1));
    the exp row-sum rides the PV matmul as a ones-column of v (M=65).
  - Output projection, writing its own [1024, 1024] slice of the output.

All matmuls run as float32r (FP22 multiplies, FP32 accumulate) which streams
at 1 cycle/row when the moving free dim is >= 256.

Weights are passed to the device pre-transposed ([d_in, d_out]) - a host-side
layout choice; activations are transposed on-device with PE transpose_mode.
"""

import sys
import types

import numpy as np

B, L, D, H, DK = 4, 2048, 1024, 16, 64
LQ = L // 2          # queries per core
P = 128              # partitions
DCH = D // P         # 8 d_in chunks
NPAIR = H // 2       # 8 head pairs
N_CORES = 8
SCALE = 1.0 / np.sqrt(DK)

_NC_CACHE = {}
DEBUG_OUTPUTS = False


def _ensure_axon_hooks():
    """Register the NTFF profile hook module if the image's antenv lacks it.

    Harmless when tracing is never requested; required for trace=True runs.
    """
    try:
        import antenv  # noqa: F401
        from antenv import axon_hooks  # noqa: F401
        return
    except ImportError:
        pass
    try:
        import antenv

        mod = types.ModuleType("antenv.axon_hooks")
        mod._HOOK = None

        def set_axon_ntff_profile_hook(h):
            mod._HOOK = h

        def get_axon_ntff_profile_hook():
            return mod._HOOK

        mod.set_axon_ntff_profile_hook = set_axon_ntff_profile_hook
        mod.get_axon_ntff_profile_hook = get_axon_ntff_profile_hook
        sys.modules["antenv.axon_hooks"] = mod
        antenv.axon_hooks = mod
        from trn_agent_boot.trn_boot import _ntff_profile_via_ctypes

        set_axon_ntff_profile_hook(
            _ntff_profile_via_ctypes("/opt/axon/libaxon_pjrt.so")
        )
    except Exception:
        pass


def build_nc():
    import concourse.bass as bass
    import concourse.tile as tile
    from concourse import bacc, mybir
    from concourse.masks import make_identity
    from contextlib import ExitStack

    f32 = mybir.dt.float32
    f32r = mybir.dt.float32r
    Exp = mybir.ActivationFunctionType.Exp

    def r(ap):
        return ap.bitcast(f32r)

    nc = bacc.Bacc(
        "TRN2",
        target_bir_lowering=False,
        debug=False,
        enable_asserts=False,
        num_devices=N_CORES,
    )

    Qc = nc.dram_tensor("Qc", [LQ, D], f32, kind="ExternalInput").ap()
    Kc = nc.dram_tensor("Kc", [L, D], f32, kind="ExternalInput").ap()
    Vc = nc.dram_tensor("Vc", [L, D], f32, kind="ExternalInput").ap()
    WQT = nc.dram_tensor("WQT", [D, D], f32, kind="ExternalInput").ap()
    WKT = nc.dram_tensor("WKT", [D, D], f32, kind="ExternalInput").ap()
    WVT = nc.dram_tensor("WVT", [D, D], f32, kind="ExternalInput").ap()
    WOT = nc.dram_tensor("WOT", [D, D], f32, kind="ExternalInput").ap()
    Yc = nc.dram_tensor("Yc", [LQ, D], f32, kind="ExternalOutput").ap()
    dbg = {}
    if DEBUG_OUTPUTS:
        dbg["qT"] = nc.dram_tensor("dbg_qT", [D, LQ], f32, kind="ExternalOutput").ap()
        dbg["kT"] = nc.dram_tensor("dbg_kT", [D, L], f32, kind="ExternalOutput").ap()
        dbg["v"] = nc.dram_tensor("dbg_v", [L, NPAIR * 130], f32, kind="ExternalOutput").ap()
        dbg["oT"] = nc.dram_tensor("dbg_oT", [D, LQ], f32, kind="ExternalOutput").ap()
        dbg["tmp"] = nc.dram_tensor("dbg_tmp", [NPAIR * 2 * 2 * 65, 512], f32, kind="ExternalOutput").ap()
        dbg["bc"] = nc.dram_tensor("dbg_bc", [NPAIR * 2 * 2 * 64, 512], f32, kind="ExternalOutput").ap()
        dbg["e"] = nc.dram_tensor("dbg_e", [16 * P, 1024], f32, kind="ExternalOutput").ap()

    with tile.TileContext(nc) as tc, ExitStack() as top:
        const = top.enter_context(tc.tile_pool(name="const", bufs=1))
        ident0 = const.tile([P, P], f32)
        make_identity(nc, ident0[:])
        ident = const.tile([P, P], f32)
        nc.vector.tensor_copy(r(ident[:]), ident0[:])

        dram = top.enter_context(tc.tile_pool(name="dram", bufs=1, space="DRAM"))
        qT_d = dram.tile([D, LQ], f32)       # q^T: [d_out, lq]
        kT_d = dram.tile([D, L], f32)        # k^T: [d_out, lk]
        v_d = dram.tile([L, NPAIR * 130], f32)  # v: [lk, pair-blocks of A|1|B|1]

        wpool = top.enter_context(tc.tile_pool(name="w", bufs=2))

        def load_weight(WT):
            w = wpool.tile([P, DCH * D], f32)
            nc.sync.dma_start(
                r(w[:].rearrange("p (c o) -> p c o", c=DCH)),
                r(WT.rearrange("(c p) o -> p c o", p=P)),
            )
            return w

        # ---------------- Phase A: transposes + projections ----------------
        with ExitStack() as pa:
            xin = pa.enter_context(tc.tile_pool(name="xin", bufs=3))
            xtp = pa.enter_context(tc.tile_pool(name="xt", bufs=2))
            stg = pa.enter_context(tc.tile_pool(name="stg", bufs=4))
            tpsum = pa.enter_context(
                tc.tile_pool(name="tpsum", bufs=2, space="PSUM")
            )
            ppsum = pa.enter_context(
                tc.tile_pool(name="ppsum", bufs=4, space="PSUM")
            )

            def transpose_half(X, l0, nlt):
                """Return xt tile [P, DCH*lhalf]: xt[p, c*lhalf + j] = X[l0+j, c*P+p]."""
                lhalf = nlt * P
                xt = xtp.tile([P, DCH * lhalf], f32)
                xt3 = xt[:].rearrange("p (c l) -> p c l", c=DCH)
                for i in range(nlt):
                    xi = xin.tile([P, D], f32)
                    nc.sync.dma_start(
                        r(xi[:]), r(X[l0 + P * i : l0 + P * (i + 1), :])
                    )
                    for cg in range(2):
                        pt = tpsum.tile([P, 512], f32)
                        for j in range(4):
                            c = 4 * cg + j
                            nc.tensor.matmul(
                                r(pt[:, P * j : P * (j + 1)]),
                                r(xi[:, P * c : P * (c + 1)]),
                                r(ident[:]),
                                is_transpose=True,
                                start=True,
                                stop=True,
                            )
                        nc.vector.tensor_copy(
                            r(xt3[:, 4 * cg : 4 * cg + 4, P * i : P * (i + 1)]),
                            pt[:].rearrange("p (c l) -> p c l", c=4),
                        )
                return xt

            # V phase: natural-layout v with interleaved ones columns
            wv = load_weight(WVT)
            for half in range(2):
                l0 = half * (L // 2)
                xt = transpose_half(Vc, l0, 8)
                xt3 = xt[:].rearrange("p (c l) -> p c l", c=DCH)
                for i in range(8):
                    for nb in range(2):
                        pp = ppsum.tile([P, 512], f32)
                        for c in range(DCH):
                            nc.tensor.matmul(
                                pp[:],
                                r(xt3[:, c, P * i : P * (i + 1)]),
                                r(wv[:, c * D + 512 * nb : c * D + 512 * (nb + 1)]),
                                start=(c == 0),
                                stop=(c == DCH - 1),
                            )
                        vs = stg.tile([P, 520], f32)
                        vs4 = vs[:].rearrange("p (a h s) -> p a h s", h=2, s=65)
                        nc.vector.tensor_copy(
                            vs4[:, :, :, 0:64],
                            pp[:].rearrange("p (a h s) -> p a h s", h=2, s=64),
                        )
                        nc.gpsimd.memset(vs4[:, :, :, 64:65], 1.0)
                        row = l0 + P * i
                        nc.sync.dma_start(
                            v_d[row : row + P, 520 * nb : 520 * (nb + 1)], vs[:]
                        )

            # K phase: kT[d_out, l]
            wk = load_weight(WKT)
            for half in range(2):
                l0 = half * (L // 2)
                xt = transpose_half(Kc, l0, 8)
                xt3 = xt[:].rearrange("p (c l) -> p c l", c=DCH)
                for pr in range(NPAIR):
                    for lb in range(2):
                        pp = ppsum.tile([P, 512], f32)
                        for c in range(DCH):
                            nc.tensor.matmul(
                                pp[:],
                                r(wk[:, c * D + P * pr : c * D + P * (pr + 1)]),
                                r(xt3[:, c, 512 * lb : 512 * (lb + 1)]),
                                start=(c == 0),
                                stop=(c == DCH - 1),
                            )
                        ks = stg.tile([P, 512], f32)
                        nc.vector.tensor_copy(ks[:], pp[:])
                        nc.sync.dma_start(
                            kT_d[
                                P * pr : P * (pr + 1),
                                l0 + 512 * lb : l0 + 512 * (lb + 1),
                            ],
                            ks[:],
                        )

            # Q phase: qT[d_out, lq] (single half of 1024 queries)
            wq = load_weight(WQT)
            xt = transpose_half(Qc, 0, 8)
            xt3 = xt[:].rearrange("p (c l) -> p c l", c=DCH)
            for pr in range(NPAIR):
                for lb in range(2):
                    pp = ppsum.tile([P, 512], f32)
                    for c in range(DCH):
                        nc.tensor.matmul(
                            pp[:],
                            r(wq[:, c * D + P * pr : c * D + P * (pr + 1)]),
                            r(xt3[:, c, 512 * lb : 512 * (lb + 1)]),
                            start=(c == 0),
                            stop=(c == DCH - 1),
                        )
                    qs_t = stg.tile([P, 512], f32)
                    nc.vector.tensor_copy(qs_t[:], pp[:])
                    nc.sync.dma_start(
                        qT_d[P * pr : P * (pr + 1), 512 * lb : 512 * (lb + 1)],
                        qs_t[:],
                    )

        if DEBUG_OUTPUTS:
            nc.sync.dma_start(dbg["qT"], qT_d[:])
            nc.sync.dma_start(dbg["kT"], kT_d[:])
            nc.sync.dma_start(dbg["v"], v_d[:])

        # ---------------- Phase B: attention ----------------
        NKT = L // P  # 16 k-tiles
        with ExitStack() as pb:
            katt = pb.enter_context(tc.tile_pool(name="katt", bufs=2))
            qatt = pb.enter_context(tc.tile_pool(name="qatt", bufs=2))
            vatt = pb.enter_context(tc.tile_pool(name="vatt", bufs=2))
            epool = pb.enter_context(tc.tile_pool(name="epool", bufs=3))
            npool = pb.enter_context(tc.tile_pool(name="npool", bufs=4))
            rpool = pb.enter_context(tc.tile_pool(name="rpool", bufs=4))
            otp = pb.enter_context(tc.tile_pool(name="otp", bufs=NPAIR))
            spsum = pb.enter_context(
                tc.tile_pool(name="spsum", bufs=2, space="PSUM")
            )
            pvpsum = pb.enter_context(
                tc.tile_pool(name="pvpsum", bufs=4, space="PSUM")
            )

            ot_tiles = []
            for pr in range(NPAIR):
                kt = katt.tile([P, L], f32)
                nc.sync.dma_start(r(kt[:]), r(kT_d[P * pr : P * (pr + 1), :]))
                qt = qatt.tile([P, LQ], f32)
                nc.sync.dma_start(r(qt[:]), r(qT_d[P * pr : P * (pr + 1), :]))
                v1 = vatt.tile([P, NKT * 130], f32)
                nc.sync.dma_start(
                    r(v1[:].rearrange("p (t c) -> p t c", c=130)),
                    r(v_d[:].rearrange("(t p) (a c) -> p t a c", p=P, c=130)[
                        :, :, pr, :
                    ]),
                )
                ot = otp.tile([P, LQ], f32)
                ot_tiles.append(ot)

                for qs in range(2):
                    qsl = qt[:, 512 * qs : 512 * (qs + 1)]
                    pvA = pvpsum.tile([P, 512], f32, tag="pv")
                    pvB = pvpsum.tile([P, 512], f32, tag="pv")
                    for g in range(NKT):
                        sab = spsum.tile([P, 1024], f32)
                        nc.tensor.matmul(
                            sab[:, 0:512],
                            r(kt[0:64, P * g : P * (g + 1)]),
                            r(qsl[0:64, :]),
                            start=True,
                            stop=True,
                        )
                        nc.tensor.matmul(
                            sab[:, 512:1024],
                            r(kt[64:128, P * g : P * (g + 1)]),
                            r(qsl[64:128, :]),
                            start=True,
                            stop=True,
                        )
                        e = epool.tile([P, 1024], f32)
                        nc.scalar.activation(
                            r(e[:]), sab[:], Exp, scale=float(SCALE)
                        )
                        if DEBUG_OUTPUTS and pr == 0 and qs == 0:
                            nc.sync.dma_start(
                                dbg["e"][P * g : P * (g + 1), :], e[:]
                            )
                        nc.tensor.matmul(
                            pvA[0:65, :],
                            r(v1[:, 130 * g : 130 * g + 65]),
                            r(e[:, 0:512]),
                            start=(g == 0),
                            stop=(g == NKT - 1),
                        )
                        nc.tensor.matmul(
                            pvB[0:65, :],
                            r(v1[:, 130 * g + 65 : 130 * g + 130]),
                            r(e[:, 512:1024]),
                            start=(g == 0),
                            stop=(g == NKT - 1),
                        )
                    # normalize: rows 0:64 = sum(exp*v), row 64 = sum(exp)
                    for h, pv in ((0, pvA), (1, pvB)):
                        tmp = npool.tile([P, 512], f32)
                        nc.vector.tensor_copy(tmp[0:65, :], pv[0:65, :])
                        # custom DVE ops and partition_broadcast only operate
                        # from partition base 0 on HW: shift the sums row down
                        srow = rpool.tile([1, 512], f32)
                        nc.vector.tensor_copy(srow[0:1, :], tmp[64:65, :])
                        rec = rpool.tile([1, 512], f32)
                        nc.vector.reciprocal_approx_fast(
                            rec[0:1, :], srow[0:1, :]
                        )
                        bc = rpool.tile([64, 512], f32)
                        nc.gpsimd.partition_broadcast(bc[:, :], rec[0:1, :])
                        nc.vector.tensor_mul(
                            r(ot[64 * h : 64 * (h + 1), 512 * qs : 512 * (qs + 1)]),
                            tmp[0:64, :],
                            bc[:, :],
                        )
                        if DEBUG_OUTPUTS:
                            idx = (pr * 2 + qs) * 2 + h
                            nc.sync.dma_start(
                                dbg["tmp"][65 * idx : 65 * (idx + 1), :],
                                tmp[0:65, :],
                            )
                            nc.sync.dma_start(
                                dbg["bc"][64 * idx : 64 * (idx + 1), :], bc[:, :]
                            )

            if DEBUG_OUTPUTS:
                for pr in range(NPAIR):
                    nc.sync.dma_start(
                        dbg["oT"][P * pr : P * (pr + 1), :], ot_tiles[pr][:]
                    )

            # ---------------- Phase C: output projection ----------------
            wo = load_weight(WOT)
            ystg = pb.enter_context(tc.tile_pool(name="ystg", bufs=3))
            for qt_i in range(LQ // P):
                for nb in range(2):
                    pp = spsum.tile([P, 512], f32, tag="sab")
                    for pr in range(NPAIR):
                        nc.tensor.matmul(
                            pp[:],
                            r(ot_tiles[pr][:, P * qt_i : P * (qt_i + 1)]),
                            r(wo[:, pr * D + 512 * nb : pr * D + 512 * (nb + 1)]),
                            start=(pr == 0),
                            stop=(pr == NPAIR - 1),
                        )
                    ys = ystg.tile([P, 512], f32)
                    nc.vector.tensor_copy(ys[:], pp[:])
                    nc.sync.dma_start(
                        Yc[P * qt_i : P * (qt_i + 1), 512 * nb : 512 * (nb + 1)],
                        ys[:],
                    )

    nc.compile()
    return nc


def get_nc():
    if "nc" not in _NC_CACHE:
        _NC_CACHE["nc"] = build_nc()
    return _NC_CACHE["nc"]


def make_in_maps(Q, K, V, WQ, WK, WV, WO):
    Q = np.asarray(Q, dtype=np.float32)
    K = np.asarray(K, dtype=np.float32)
    V = np.asarray(V, dtype=np.float32)
    WQT = np.ascontiguousarray(np.asarray(WQ, dtype=np.float32).T)
    WKT = np.ascontiguousarray(np.asarray(WK, dtype=np.float32).T)
    WVT = np.ascontiguousarray(np.asarray(WV, dtype=np.float32).T)
    WOT = np.ascontiguousarray(np.asarray(WO, dtype=np.float32).T)
    in_maps = []
    for c in range(N_CORES):
        b, half = c // 2, c % 2
        in_maps.append(
            {
                "Qc": np.ascontiguousarray(Q[b, half * LQ : (half + 1) * LQ, :]),
                "Kc": np.ascontiguousarray(K[b]),
                "Vc": np.ascontiguousarray(V[b]),
                "WQT": WQT,
                "WKT": WKT,
                "WVT": WVT,
                "WOT": WOT,
            }
        )
    return in_maps


def kernel(Q, K, V, WQ, WK, WV, WO, trace=False):
    _ensure_axon_hooks()
    from concourse.bass_utils import run_bass_kernel_spmd

    nc = get_nc()
    in_maps = make_in_maps(Q, K, V, WQ, WK, WV, WO)
    res = run_bass_kernel_spmd(
        nc, in_maps, core_ids=list(range(N_CORES)), trace=trace
    )
    out = np.empty((B, L, D), dtype=np.float32)
    for c in range(N_CORES):
        b, half = c // 2, c % 2
        out[b, half * LQ : (half + 1) * LQ, :] = res.results[c]["Yc"]
    if trace:
        kernel.last_results = res
    return out
